# revision 15
# baseline (speedup 1.0000x reference)
"""EntitySelector sparse-attention kernel for 8 Trainium2 NeuronCores.

Sharding: data-parallel over batch (16 batches -> 2 per core), replicated
(host-folded) weights, per-batch entity gather via indirect DMA.

Algorithmic reassociation (the big win vs the direct formulation): the Q
and WO projections over L=1024 query rows are folded onto the entity side
(NB=256 rows) using host-precomputed fused matrices:

  K'  = ent_g @ M1,  M1 = WK^T @ WQ          (scores key)
  s_n = ent_g @ m3,  m3 = WK^T @ bq          (per-entity score offset)
  scores[l,n] = query[l] . K'[n] + s_n       (raw query, no Q projection!)
  V'' = ent_g @ M2 + sqrt(D)*bo', M2 = WK^T @ WO^T,
        bo' = bo + D^-0.5 (bk @ WO^T)        (WO folded into values;
                                              bias via sum(probs)=D^-0.5)
  out = probs @ V''  -> LayerNorm            (no WO projection!)

The WQ bias and constant terms are softmax-invariant and dropped; padding
is handled by multiplying probs with the mask post-exp (exactly equivalent
to the reference's -1e5 additive mask after renormalization).

All matmuls run in bf16 (validated 6.0e-3 rel err vs 2e-2 gate); softmax
statistics and LayerNorm run in fp32.
"""

import sys

sys.path.insert(0, "/opt/trn_rl_repo")

import numpy as np
import ml_dtypes

import concourse.bass as bass
import concourse.mybir as mybir
import concourse.tile as tile
from concourse.tile_rust import add_dep_helper
from concourse import bacc
from concourse.bass_utils import run_bass_kernel_spmd
from concourse.masks import make_identity

P = 128
D = 1024
DT = D // P            # 8 feature tiles
B = 16
BL = 2                 # batches per core
L = 1024
LC = 512               # l-chunk (query stream granularity)
NLC = L // LC          # 2 chunks
LT = LC // P           # 4 l-tiles per chunk
NB = 256
NT = NB // P           # 2 entity tiles
NE = 20000
NCORES = 8

F32 = mybir.dt.float32
BF16 = mybir.dt.bfloat16
I32 = mybir.dt.int32

AF = mybir.ActivationFunctionType
OP = mybir.AluOpType
AX = mybir.AxisListType

_CACHE = {}


class _Ctx:
    pass


def _emit_gather(nc, g, b):
    """Indirect-gather this batch's entities (issue early: SWDGE ring FIFO)."""
    idx_col = g.bpool.tile([P, NT], I32, tag="idxc")
    nc.gpsimd.dma_start(idx_col, g.idx[b].rearrange("(t p) -> p t", p=P))
    # per-entity additive mask row: 0 for real, -1e5 for padding (host-folded)
    mb_row = g.bpool.tile([1, NB], F32, tag="mbr")
    mrow = g.msk[b]
    nc.gpsimd.dma_start(
        mb_row,
        bass.AP(tensor=mrow.tensor, offset=mrow.offset,
                ap=[[0, 1]] + list(mrow.ap)))

    ent_sb = []
    g_insts = []
    for nt in range(NT):
        e = g.entp.tile([P, D], F32, tag="ent")
        gi = nc.gpsimd.indirect_dma_start(
            out=e[:], out_offset=None, in_=g.emb[:, :],
            in_offset=bass.IndirectOffsetOnAxis(ap=idx_col[:, nt:nt + 1], axis=0))
        g_insts.append(gi)
        ent_sb.append(e)
    return ent_sb, mb_row, g_insts


def _emit_entity_stage(nc, g, b, ent_sb, mb_row):
    """Per-batch entity-side compute: entT, K', s, s_bc, V''."""
    # cast gathered entities to bf16 (gpsimd; transposes run 2x faster on bf16)
    entb = g.entbp.tile([P, NT, D], BF16, tag="entb")
    for nt in range(NT):
        nc.gpsimd.tensor_copy(entb[:, nt, :], ent_sb[nt][:])

    # entT: feature-major bf16 entities [di, n]
    entT = g.enttp.tile([P, DT, NB], BF16, tag="entT")
    for dt in range(DT):
        pt = g.ps_tr.tile([P, NB], BF16, tag="ptr")
        for nt in range(NT):
            nc.tensor.transpose(pt[:, nt * P:(nt + 1) * P],
                                entb[:, nt, dt * P:(dt + 1) * P], g.ident_b)
        nc.scalar.mul(entT[:, dt, :], pt, 1.0)

    # K' = ent_g @ M1, feature-major [do, n]
    kt_sb = g.ktp.tile([P, DT, NB], BF16, tag="kt")
    for do in range(DT):
        pk = g.ps_a.tile([P, NB], F32, tag="psc")
        for kt in range(DT):
            nc.tensor.matmul(pk, g.m1_sb[:, kt, do * P:(do + 1) * P],
                             entT[:, kt, :],
                             start=(kt == 0), stop=(kt == DT - 1))
        nc.scalar.mul(kt_sb[:, do, :], pk, 1.0)

    # s = ent_g @ m3 (1-row psum) + additive pad-mask, rank-1 bcast to [P, NB]
    ps1 = g.ps_tr.tile([1, NB], F32, tag="ptr", name="ps1")
    for kt in range(DT):
        nc.tensor.matmul(ps1, g.m3_col[:, kt:kt + 1], entT[:, kt, :],
                         start=(kt == 0), stop=(kt == DT - 1))
    s_sb = g.ssp.tile([1, NB], BF16, tag="ssb")
    nc.vector.tensor_add(s_sb, ps1, mb_row)
    pbc = g.ps_a.tile([P, NB], F32, tag="psc")
    nc.tensor.matmul(pbc, g.ones_b, s_sb, start=True, stop=True)
    s_bc = g.sbcp.tile([P, NB], F32, tag="sbc")
    nc.vector.tensor_copy(s_bc, pbc)

    # V'' = ent_g @ M2 + sqrt(D)*bo' (entity-major bf16 [n, do])
    vem = g.kp.tile([P, NT, D], BF16, tag="vem")
    for nt in range(NT):
        for half in range(2):
            pv = g.ps_b.tile([P, LC], F32, tag="pbig")
            for kt in range(DT):
                nc.tensor.matmul(pv, entT[:, kt, nt * P:(nt + 1) * P],
                                 g.m2_sb[:, kt, half * LC:(half + 1) * LC],
                                 start=(kt == 0), stop=(kt == DT - 1))
            nc.vector.tensor_add(vem[:, nt, half * LC:(half + 1) * LC],
                                 pv, g.vb_bc[:, half * LC:(half + 1) * LC])
    return kt_sb, s_bc, vem


def _emit_scores(nc, g, qin, t, kt_sb):
    """Scores matmul for one l-tile (psum, pre-softmax)."""
    psc = g.ps_a.tile([P, NB], F32, tag="psc")
    for dt in range(DT):
        nc.tensor.matmul(psc, qin[:, dt, t * P:(t + 1) * P],
                         kt_sb[:, dt, :],
                         start=(dt == 0), stop=(dt == DT - 1))
    return psc


def _emit_softmax(nc, g, psc, s_bc):
    """exp(scores + s - max): unnormalized — LayerNorm's scale invariance
    cancels the softmax denominator (incl. the D^-0.5 factor) exactly;
    padding columns carry -1e5 via s_bc so exp underflows to +0."""
    nc.vector.tensor_add(psc, psc, s_bc)  # += per-entity offset & pad mask
    negmax = g.lnp.tile([P, 1], F32, tag="nm")
    nc.vector.reduce_max(negmax, psc, axis=AX.X, negate=True)
    probs = g.probsp.tile([P, NB], BF16, tag="probs")
    nc.scalar.activation(out=probs, in_=psc, func=AF.Exp, bias=negmax, scale=1.0)
    ptb = g.ps_tr.tile([P, NB], BF16, tag="ptr")
    for nt in range(NT):
        nc.tensor.transpose(ptb[:, nt * P:(nt + 1) * P],
                            probs[:, nt * P:(nt + 1) * P], g.ident_b)
    pT = g.ptp.tile([P, NT, P], BF16, tag="pT")
    nc.vector.tensor_copy(pT, ptb.rearrange("p (a b) -> p a b", a=NT))
    return pT


def _emit_out_tile(nc, g, b, lt, pT, vem):
    """PV matmul + LayerNorm + output DMA for one l-tile."""
    stats = g.lnp.tile([P, 2, 6], F32, tag="stats")
    ppv = []
    for half in range(2):
        po = g.ps_b.tile([P, LC], F32, tag="pbig")
        for nt in range(NT):
            nc.tensor.matmul(po, pT[:, nt, :],
                             vem[:, nt, half * LC:(half + 1) * LC],
                             start=(nt == 0), stop=(nt == NT - 1))
        nc.vector.bn_stats(out=stats[:, half, :], in_=po)
        ppv.append(po)

    mv = g.lnp.tile([P, 2], F32, tag="mv")
    nc.vector.bn_aggr(out=mv, in_=stats)
    # rstd = exp(-0.5*ln(var+eps)): Ln/Exp/Copy/Identity share one ACT
    # table (natural_log_exp_and_others) -> no act-table reloads
    lnv = g.lnp.tile([P, 1], F32, tag="lnv")
    nc.scalar.activation(out=lnv, in_=mv[:, 1:2], func=AF.Ln,
                         bias=g.eps_t, scale=1.0)
    rstd = g.lnp.tile([P, 1], F32, tag="rstd")
    nc.scalar.activation(out=rstd, in_=lnv, func=AF.Exp, bias=0.0, scale=-0.5)
    negmu = g.lnp.tile([P, 1], F32, tag="nmu")
    nc.scalar.mul(negmu, mv[:, 0:1], -1.0)
    nmr = g.lnp.tile([P, 1], F32, tag="nmr")
    nc.vector.tensor_mul(nmr, negmu, rstd)

    o_sb = g.opool.tile([P, D], F32, tag="o")
    for half in range(2):
        nc.scalar.activation(out=o_sb[:, half * LC:(half + 1) * LC],
                             in_=ppv[half], func=AF.Identity,
                             bias=nmr, scale=rstd)
    if g.apply_affine:
        nc.vector.tensor_mul(o_sb, o_sb, g.lng_bc)
        nc.vector.tensor_add(o_sb, o_sb, g.lnb_bc)
    nc.scalar.dma_start(g.out[b, lt * P:(lt + 1) * P, :], o_sb)


def _emit_batch(nc, g, b, ent_sb, mb_row, post_qin_hook=None):
    kt_sb, s_bc, vem = _emit_entity_stage(nc, g, b, ent_sb, mb_row)
    qTb = g.qT[b].rearrange("(kt p) l -> p kt l", p=P)
    for lc in range(NLC):
        qin = g.qinp.tile([P, DT, LC], BF16, tag="qin")
        qin_i = nc.sync.dma_start(qin, qTb[:, :, lc * LC:(lc + 1) * LC])
        if b == 0 and lc == 0 and g.w_last is not None:
            add_dep_helper(qin_i.ins, g.w_last.ins,
                           reason="first query chunk after weights")
        if b == 0 and lc == 0 and post_qin_hook is not None:
            post_qin_hook(qin_i)
        if b == 0 and lc == 1 and getattr(g, "g1_insts", None):
            for gi in g.g1_insts:
                add_dep_helper(qin_i.ins, gi.ins,
                               reason="2nd query chunk after b1 gather")

        # software-pipelined l-tiles: scores run 2 tiles ahead of PV/LN
        pend = []
        for t in range(LT):
            psc = _emit_scores(nc, g, qin, t, kt_sb)
            pend.append(psc)
            if t >= 2:
                pT = _emit_softmax(nc, g, pend[t - 2], s_bc)
                _emit_out_tile(nc, g, b, lc * LT + (t - 2), pT, vem)
        for t in (LT - 2, LT - 1):
            pT = _emit_softmax(nc, g, pend[t], s_bc)
            _emit_out_tile(nc, g, b, lc * LT + t, pT, vem)


def build_nc(apply_affine):
    nc = bacc.Bacc("TRN2", target_bir_lowering=False, debug=False,
                   num_devices=NCORES)
    g = _Ctx()
    g.apply_affine = apply_affine

    g.qT = nc.dram_tensor("qT", [BL, D, L], BF16, kind="ExternalInput")
    g.emb = nc.dram_tensor("emb", [NE, D], F32, kind="ExternalInput")
    g.idx = nc.dram_tensor("idx", [BL, NB], I32, kind="ExternalInput")
    g.msk = nc.dram_tensor("msk", [BL, NB], F32, kind="ExternalInput")
    m1 = nc.dram_tensor("m1", [D, D], BF16, kind="ExternalInput")
    m2 = nc.dram_tensor("m2", [D, D], BF16, kind="ExternalInput")
    m3 = nc.dram_tensor("m3", [D], BF16, kind="ExternalInput")
    vb = nc.dram_tensor("vb", [D], F32, kind="ExternalInput")
    if apply_affine:
        lng = nc.dram_tensor("lng", [D], F32, kind="ExternalInput")
        lnb = nc.dram_tensor("lnb", [D], F32, kind="ExternalInput")
    g.out = nc.dram_tensor("out", [BL, L, D], F32, kind="ExternalOutput")

    def bcast_row(dram_1d):
        ap = dram_1d[:]
        return bass.AP(tensor=ap.tensor, offset=ap.offset,
                       ap=[[0, P]] + list(ap.ap))

    with tile.TileContext(nc) as tc:
        with (
            tc.tile_pool(name="wpool", bufs=1) as wpool,
            tc.tile_pool(name="bpool", bufs=2) as bpool,
            tc.tile_pool(name="entp", bufs=3) as entp,
            tc.tile_pool(name="entb", bufs=2) as entbp,
            tc.tile_pool(name="entt", bufs=2) as enttp,
            tc.tile_pool(name="ktp", bufs=2) as ktp,
            tc.tile_pool(name="kp", bufs=2) as kp,
            tc.tile_pool(name="qinp", bufs=2) as qinp,
            tc.tile_pool(name="probsp", bufs=3) as probsp,
            tc.tile_pool(name="ptp", bufs=3) as ptp,
            tc.tile_pool(name="sbcp", bufs=2) as sbcp,
            tc.tile_pool(name="ssp", bufs=2) as ssp,
            tc.tile_pool(name="op", bufs=3) as opool,
            tc.tile_pool(name="lnp", bufs=4) as lnp,
            tc.tile_pool(name="ps_a", bufs=3, space="PSUM") as ps_a,
            tc.tile_pool(name="ps_b", bufs=3, space="PSUM") as ps_b,
            tc.tile_pool(name="ps_tr", bufs=2, space="PSUM") as ps_tr,
        ):
            g.bpool, g.entp, g.entbp, g.enttp = bpool, entp, entbp, enttp
            g.ktp, g.kp, g.qinp, g.probsp, g.ptp = ktp, kp, qinp, probsp, ptp
            g.sbcp, g.ssp, g.opool, g.lnp = sbcp, ssp, opool, lnp
            g.ps_a, g.ps_b, g.ps_tr = ps_a, ps_b, ps_tr

            ident = wpool.tile([P, P], F32)
            make_identity(nc, ident)
            g.ident_b = wpool.tile([P, P], BF16)
            nc.vector.tensor_copy(g.ident_b, ident)
            g.eps_t = wpool.tile([P, 1], F32)
            nc.vector.memset(g.eps_t, 1e-5)
            g.ones_b = wpool.tile([1, P], BF16)
            nc.vector.memset(g.ones_b, 1.0)

            # batch-0 gathers go first on the SWDGE ring
            ent0, mbr0, g0_insts = _emit_gather(nc, g, 0)

            # weights: m1 first (K' needs it earliest), then m2, in column
            # halves so consumers can start after half arrives
            g.m1_sb = wpool.tile([P, DT, D], BF16)
            m1_r = m1[:, :].rearrange("(kt p) m -> p kt m", p=P)
            g.m2_sb = wpool.tile([P, DT, D], BF16)
            m2_r = m2[:, :].rearrange("(kt p) m -> p kt m", p=P)
            bulk = []
            for h in range(2):
                bulk.append(nc.scalar.dma_start(
                    g.m1_sb[:, :, h * LC:(h + 1) * LC],
                    m1_r[:, :, h * LC:(h + 1) * LC]))
            for h in range(2):
                bulk.append(nc.scalar.dma_start(
                    g.m2_sb[:, :, h * LC:(h + 1) * LC],
                    m2_r[:, :, h * LC:(h + 1) * LC]))
            for bi in bulk:
                for gi in g0_insts:
                    add_dep_helper(bi.ins, gi.ins,
                                   reason="bulk weight load after b0 gather")
            g.w_last = bulk[-1]

            g.m3_col = wpool.tile([P, DT], BF16)
            nc.scalar.dma_start(g.m3_col, m3[:].rearrange("(t p) -> p t", p=P))
            g.vb_bc = wpool.tile([P, D], F32)
            nc.scalar.dma_start(g.vb_bc, bcast_row(vb))
            if apply_affine:
                g.lng_bc = wpool.tile([P, D], F32)
                nc.scalar.dma_start(g.lng_bc, bcast_row(lng))
                g.lnb_bc = wpool.tile([P, D], F32)
                nc.scalar.dma_start(g.lnb_bc, bcast_row(lnb))

            state = {}

            def post_qin_hook(qin_i):
                # b1 gathers: after first query chunk on the DMA resource
                ent1, mbr1, g1_insts = _emit_gather(nc, g, 1)
                for gi in g1_insts:
                    add_dep_helper(gi.ins, qin_i.ins,
                                   reason="b1 gather after first query chunk")
                g.g1_insts = g1_insts
                state["b1"] = (ent1, mbr1)

            _emit_batch(nc, g, 0, ent0, mbr0, post_qin_hook=post_qin_hook)
            ent1, mbr1 = state["b1"]
            _emit_batch(nc, g, 1, ent1, mbr1)

    nc.compile()
    return nc


def _get_nc(apply_affine):
    key = bool(apply_affine)
    if key not in _CACHE:
        _CACHE[key] = build_nc(key)
    return _CACHE[key]


def kernel(query, ent_emb, ent_idx_in_batch, max_entity_number,
           WQ_w, WQ_b, WK_w, WK_b, WO_w, WO_b, ln_g, ln_b):
    query = np.asarray(query, np.float32)
    ent_emb = np.ascontiguousarray(np.asarray(ent_emb, np.float32))
    idx = np.asarray(ent_idx_in_batch)
    # additive pad-mask row: 0 for real entities, -1e5 for -1 padding
    mbias = ((idx != -1).astype(np.float32) - 1.0) * 1.0e5
    idx32 = np.where(idx < 0, 0, idx).astype(np.int32)
    wq = np.asarray(WQ_w, np.float32)
    wk = np.asarray(WK_w, np.float32)
    wo = np.asarray(WO_w, np.float32)
    bq = np.asarray(WQ_b, np.float32)
    bk = np.asarray(WK_b, np.float32)
    bo = np.asarray(WO_b, np.float32)
    lng = np.asarray(ln_g, np.float32)
    lnb = np.asarray(ln_b, np.float32)
    apply_affine = not (np.all(lng == 1.0) and np.all(lnb == 0.0))

    # host-folded fused weights (see module docstring)
    wkT = np.ascontiguousarray(wk.T)
    m1 = (wkT @ wq).astype(ml_dtypes.bfloat16)
    m2 = (wkT @ wo.T).astype(ml_dtypes.bfloat16)
    m3 = (wkT @ bq).astype(ml_dtypes.bfloat16)
    vb = (np.sqrt(float(D)) * bo + (bk @ wo.T)).astype(np.float32)

    qT = np.ascontiguousarray(
        query.transpose(0, 2, 1)).astype(ml_dtypes.bfloat16)  # (B, D, L)

    nc = _get_nc(apply_affine)
    in_maps = []
    for c in range(NCORES):
        s = slice(c * BL, (c + 1) * BL)
        m = dict(
            qT=np.ascontiguousarray(qT[s]),
            emb=ent_emb,
            idx=np.ascontiguousarray(idx32[s]),
            msk=np.ascontiguousarray(mbias[s]),
            m1=m1, m2=m2, m3=m3, vb=vb,
        )
        if apply_affine:
            m["lng"] = lng
            m["lnb"] = lnb
        in_maps.append(m)

    res = run_bass_kernel_spmd(nc, in_maps, core_ids=list(range(NCORES)))
    return np.concatenate([r["out"] for r in res.results], axis=0)


# revision 28
# speedup vs baseline: 1.1411x; 1.1411x over previous
"""EntitySelector sparse-attention kernel for 8 Trainium2 NeuronCores.

Sharding: data-parallel over batch (16 batches -> 2 per core), replicated
(host-folded) weights, per-batch entity gather via indirect DMA.

Algorithmic reassociation (the big win vs the direct formulation): the Q
and WO projections over L=1024 query rows are folded onto the entity side
(NB=256 rows) using host-precomputed fused matrices:

  K'  = ent_g @ M1,  M1 = WK^T @ WQ          (scores key)
  s_n = ent_g @ m3,  m3 = WK^T @ bq          (per-entity score offset)
  scores[l,n] = query[l] . K'[n] + s_n       (raw query, no Q projection!)
  V'' = ent_g @ M2 + sqrt(D)*bo', M2 = WK^T @ WO^T,
        bo' = bo + D^-0.5 (bk @ WO^T)        (WO folded into values;
                                              bias via sum(probs)=D^-0.5)
  out = probs @ V''  -> LayerNorm            (no WO projection!)

The WQ bias and constant terms are softmax-invariant and dropped; padding
is handled by multiplying probs with the mask post-exp (exactly equivalent
to the reference's -1e5 additive mask after renormalization).

All matmuls run in bf16 (validated 6.0e-3 rel err vs 2e-2 gate); softmax
statistics and LayerNorm run in fp32.
"""

import sys

sys.path.insert(0, "/opt/trn_rl_repo")

import numpy as np
import ml_dtypes

import concourse.bass as bass
import concourse.mybir as mybir
import concourse.tile as tile
from concourse.tile_rust import add_dep_helper
from concourse import bacc
from concourse.bass_utils import run_bass_kernel_spmd
from concourse.masks import make_identity

P = 128
D = 1024
DT = D // P            # 8 feature tiles
B = 16
BL = 2                 # batches per core
L = 1024
LC = 512               # l-chunk (query stream granularity)
NLC = L // LC          # 2 chunks
LT = LC // P           # 4 l-tiles per chunk
NB = 256
NT = NB // P           # 2 entity tiles
NE = 20000
NCORES = 8

F32 = mybir.dt.float32
BF16 = mybir.dt.bfloat16
I32 = mybir.dt.int32

AF = mybir.ActivationFunctionType
OP = mybir.AluOpType
AX = mybir.AxisListType

_CACHE = {}


class _Ctx:
    pass


def _emit_gather(nc, g, b):
    """Indirect-gather this batch's entities (issue early: SWDGE ring FIFO)."""
    idx_col = g.bpool.tile([P, NT], I32, tag="idxc")
    nc.gpsimd.dma_start(idx_col, g.idx[b].rearrange("(t p) -> p t", p=P))
    # per-entity additive mask row: 0 for real, -1e5 for padding (host-folded)
    mb_row = g.bpool.tile([1, NB], F32, tag="mbr")
    mrow = g.msk[b]
    nc.gpsimd.dma_start(
        mb_row,
        bass.AP(tensor=mrow.tensor, offset=mrow.offset,
                ap=[[0, 1]] + list(mrow.ap)))

    ent_sb = []
    g_insts = []
    for nt in range(NT):
        e = g.entp.tile([P, D], F32, tag="ent")
        gi = nc.gpsimd.indirect_dma_start(
            out=e[:], out_offset=None, in_=g.emb[:, :],
            in_offset=bass.IndirectOffsetOnAxis(ap=idx_col[:, nt:nt + 1], axis=0))
        g_insts.append(gi)
        ent_sb.append(e)
    return ent_sb, mb_row, g_insts


def _emit_entity_stage(nc, g, b, ent_sb, mb_row):
    """Per-batch entity-side compute: entT, K', s, s_bc, V''."""
    # cast gathered entities to bf16 (gpsimd; transposes run 2x faster on bf16)
    entb = g.entbp.tile([P, NT, D], BF16, tag="entb")
    for nt in range(NT):
        nc.gpsimd.tensor_copy(entb[:, nt, :], ent_sb[nt][:])

    # entT: feature-major bf16 entities [di, n]
    entT = g.enttp.tile([P, DT, NB], BF16, tag="entT")
    for dt in range(DT):
        pt = g.ps_tr.tile([P, NB], BF16, tag="ptr")
        for nt in range(NT):
            nc.tensor.transpose(pt[:, nt * P:(nt + 1) * P],
                                entb[:, nt, dt * P:(dt + 1) * P], g.ident_b)
        nc.scalar.mul(entT[:, dt, :], pt, 1.0)

    # K' = ent_g @ M1, feature-major [do, n]
    kt_sb = g.ktp.tile([P, DT, NB], BF16, tag="kt")
    for do in range(DT):
        pk = g.ps_a.tile([P, NB], F32, tag="psc")
        for kt in range(DT):
            nc.tensor.matmul(pk, g.m1_sb[:, kt, do * P:(do + 1) * P],
                             entT[:, kt, :],
                             start=(kt == 0), stop=(kt == DT - 1))
        nc.scalar.mul(kt_sb[:, do, :], pk, 1.0)

    # s = ent_g @ m3 (1-row psum) + additive pad-mask; consumed as the
    # rank-1 PSUM initializer of every scores matmul (no broadcast tile)
    ps1 = g.ps_tr.tile([1, NB], F32, tag="ptr", name="ps1")
    for kt in range(DT):
        nc.tensor.matmul(ps1, g.m3_col[:, kt:kt + 1], entT[:, kt, :],
                         start=(kt == 0), stop=(kt == DT - 1))
    s_sb = g.ssp.tile([1, NB], BF16, tag="ssb")
    nc.vector.tensor_add(s_sb, ps1, mb_row)

    # V'' = ent_g @ M2 + sqrt(D)*bo' (entity-major bf16 [n, do])
    vem = g.kp.tile([P, NT, D], BF16, tag="vem")
    for nt in range(NT):
        for half in range(2):
            pv = g.ps_b.tile([P, LC], F32, tag="pbig")
            for kt in range(DT):
                nc.tensor.matmul(pv, entT[:, kt, nt * P:(nt + 1) * P],
                                 g.m2_sb[:, kt, half * LC:(half + 1) * LC],
                                 start=(kt == 0), stop=(kt == DT - 1))
            nc.vector.tensor_add(vem[:, nt, half * LC:(half + 1) * LC],
                                 pv, g.vb_bc[:, half * LC:(half + 1) * LC])
    return kt_sb, s_sb, vem


def _emit_scores(nc, g, qin, t, kt_sb, s_sb):
    """Scores matmul for one l-tile: PSUM initialized with the rank-1
    broadcast of s (per-entity offset + pad mask), then 8 qK' accums."""
    psc = g.ps_a.tile([P, NB], F32, tag="psc")
    nc.tensor.matmul(psc, g.ones_b, s_sb, start=True, stop=False)
    for dt in range(DT):
        nc.tensor.matmul(psc, qin[:, dt, t * P:(t + 1) * P],
                         kt_sb[:, dt, :],
                         start=False, stop=(dt == DT - 1))
    return psc


def _emit_softmax(nc, g, psc):
    """exp(scores - max), unnormalized — LayerNorm's scale invariance
    cancels the softmax denominator (incl. the D^-0.5 factor) exactly;
    padding columns carry -1e5 via s so exp underflows to +0. The exp's
    accum_out provides the row sum for the eps-rescale in the LN tail."""
    negmax = g.lnp.tile([P, 1], F32, tag="nm")
    nc.vector.reduce_max(negmax, psc, axis=AX.X, negate=True)
    probs = g.probsp.tile([P, NB], BF16, tag="probs")
    rsum = g.lnp.tile([P, 1], F32, tag="rs")
    nc.scalar.activation(out=probs, in_=psc, func=AF.Exp, bias=negmax,
                         scale=1.0, accum_out=rsum)
    ptb = g.ps_tr.tile([P, NB], BF16, tag="ptr")
    for nt in range(NT):
        nc.tensor.transpose(ptb[:, nt * P:(nt + 1) * P],
                            probs[:, nt * P:(nt + 1) * P], g.ident_b)
    pT = g.ptp.tile([P, NT, P], BF16, tag="pT")
    nc.vector.tensor_copy(pT, ptb.rearrange("p (a b) -> p a b", a=NT))
    return pT, rsum


def _emit_out_tile(nc, g, b, lt, pT, rsum, vem):
    """PV matmul + LayerNorm + output DMA for one l-tile.

    The output row scale is lam = rsum*sqrt(D) (unnormalized probs), so the
    reference's (var_o + 1e-5) becomes (var_raw + lam^2*1e-5). rstd is
    computed as rsqrt entirely on DVE (Newton iteration; the normalized
    w = (var_o+eps)*D lies in [0.35, 0.56] so the reciprocal seed
    converges quadratically) — no Sqrt/Ln on ACT, so ACT's function table
    stays pinned to exp_and_others with zero per-tile reloads."""
    stats = g.lnp.tile([P, 2, 6], F32, tag="stats")
    ppv = []
    for half in range(2):
        po = g.ps_b.tile([P, LC], F32, tag="pbig")
        for nt in range(NT):
            nc.tensor.matmul(po, pT[:, nt, :],
                             vem[:, nt, half * LC:(half + 1) * LC],
                             start=(nt == 0), stop=(nt == NT - 1))
        nc.vector.bn_stats(out=stats[:, half, :], in_=po)
        ppv.append(po)

    mv = g.lnp.tile([P, 2], F32, tag="mv")
    nc.vector.bn_aggr(out=mv, in_=stats)

    lt_ = g.lnp.tile([P, 1], F32, tag="t2")
    nc.vector.tensor_mul(lt_, rsum, rsum)                 # S^2
    u = g.lnp.tile([P, 1], F32, tag="u")
    nc.vector.scalar_tensor_tensor(out=u, in0=lt_, scalar=float(D) * 1e-5,
                                   in1=mv[:, 1:2], op0=OP.mult, op1=OP.add)
    rt = g.lnp.tile([P, 1], F32, tag="rt")
    nc.vector.reciprocal(rt, lt_)                         # 1/S^2
    w = g.lnp.tile([P, 1], F32, tag="w")
    nc.vector.tensor_mul(w, u, rt)                        # (var_o+eps)*D
    y = g.lnp.tile([P, 1], F32, tag="y")
    nc.vector.reciprocal(y, w)
    nc.vector.tensor_scalar(out=y, in0=y, scalar1=0.5, scalar2=0.5,
                            op0=OP.mult, op1=OP.add)      # seed (1+1/w)/2
    a2 = g.lnp.tile([P, 1], F32, tag="a2")
    c2 = g.lnp.tile([P, 1], F32, tag="c2")
    for _ in range(2):                                    # Newton rsqrt(w)
        nc.vector.tensor_mul(a2, y, y)
        nc.vector.scalar_tensor_tensor(out=c2, in0=a2, scalar=-0.5,
                                       in1=w, op0=OP.mult, op1=OP.mult)
        nc.vector.tensor_scalar_add(c2, c2, 1.5)
        nc.vector.tensor_mul(y, y, c2)
    rinv = g.lnp.tile([P, 1], F32, tag="ri")
    nc.vector.reciprocal(rinv, rsum)
    rstd = g.lnp.tile([P, 1], F32, tag="rstd")
    nc.vector.tensor_mul(rstd, y, rinv)                   # rsqrt(var+lam2 eps)
    nmr = g.lnp.tile([P, 1], F32, tag="nmr")
    nc.vector.scalar_tensor_tensor(out=nmr, in0=mv[:, 0:1], scalar=-1.0,
                                   in1=rstd, op0=OP.mult, op1=OP.mult)

    o_sb = g.opool.tile([P, D], F32, tag="o")
    for half in range(2):
        nc.scalar.activation(out=o_sb[:, half * LC:(half + 1) * LC],
                             in_=ppv[half], func=AF.Identity,
                             bias=nmr, scale=rstd)
    if g.apply_affine:
        nc.vector.tensor_mul(o_sb, o_sb, g.lng_bc)
        nc.vector.tensor_add(o_sb, o_sb, g.lnb_bc)
    nc.scalar.dma_start(g.out[b, lt * P:(lt + 1) * P, :], o_sb)


def _emit_batch(nc, g, b, ent_sb, mb_row, post_qin_hook=None):
    kt_sb, s_sb, vem = _emit_entity_stage(nc, g, b, ent_sb, mb_row)
    qTb = g.qT[b].rearrange("(kt p) l -> p kt l", p=P)
    for lc in range(NLC):
        qin = g.qinp.tile([P, DT, LC], BF16, tag="qin")
        qin_i = nc.sync.dma_start(qin, qTb[:, :, lc * LC:(lc + 1) * LC])
        if b == 0 and lc == 0 and g.w_last is not None:
            add_dep_helper(qin_i.ins, g.w_last.ins,
                           reason="first query chunk after weights")
        if b == 0 and lc == 0 and post_qin_hook is not None:
            post_qin_hook(qin_i)
        if b == 0 and lc == 1 and getattr(g, "g1_insts", None):
            for gi in g.g1_insts:
                add_dep_helper(qin_i.ins, gi.ins,
                               reason="2nd query chunk after b1 gather")

        # software-pipelined l-tiles: scores run 2 tiles ahead of PV/LN
        pend = []
        for t in range(LT):
            psc = _emit_scores(nc, g, qin, t, kt_sb, s_sb)
            pend.append(psc)
            if t >= 2:
                pT, rsum = _emit_softmax(nc, g, pend[t - 2])
                _emit_out_tile(nc, g, b, lc * LT + (t - 2), pT, rsum, vem)
        for t in (LT - 2, LT - 1):
            pT, rsum = _emit_softmax(nc, g, pend[t])
            _emit_out_tile(nc, g, b, lc * LT + t, pT, rsum, vem)


def build_nc(apply_affine):
    nc = bacc.Bacc("TRN2", target_bir_lowering=False, debug=False,
                   num_devices=NCORES)
    g = _Ctx()
    g.apply_affine = apply_affine

    g.qT = nc.dram_tensor("qT", [BL, D, L], BF16, kind="ExternalInput")
    g.emb = nc.dram_tensor("emb", [NE, D], F32, kind="ExternalInput")
    g.idx = nc.dram_tensor("idx", [BL, NB], I32, kind="ExternalInput")
    g.msk = nc.dram_tensor("msk", [BL, NB], F32, kind="ExternalInput")
    m1 = nc.dram_tensor("m1", [D, D], BF16, kind="ExternalInput")
    m2 = nc.dram_tensor("m2", [D, D], BF16, kind="ExternalInput")
    m3 = nc.dram_tensor("m3", [D], BF16, kind="ExternalInput")
    vb = nc.dram_tensor("vb", [D], F32, kind="ExternalInput")
    if apply_affine:
        lng = nc.dram_tensor("lng", [D], F32, kind="ExternalInput")
        lnb = nc.dram_tensor("lnb", [D], F32, kind="ExternalInput")
    g.out = nc.dram_tensor("out", [BL, L, D], F32, kind="ExternalOutput")

    def bcast_row(dram_1d):
        ap = dram_1d[:]
        return bass.AP(tensor=ap.tensor, offset=ap.offset,
                       ap=[[0, P]] + list(ap.ap))

    with tile.TileContext(nc) as tc:
        with (
            tc.tile_pool(name="wpool", bufs=1) as wpool,
            tc.tile_pool(name="bpool", bufs=2) as bpool,
            tc.tile_pool(name="entp", bufs=3) as entp,
            tc.tile_pool(name="entb", bufs=2) as entbp,
            tc.tile_pool(name="entt", bufs=2) as enttp,
            tc.tile_pool(name="ktp", bufs=2) as ktp,
            tc.tile_pool(name="kp", bufs=2) as kp,
            tc.tile_pool(name="qinp", bufs=2) as qinp,
            tc.tile_pool(name="probsp", bufs=3) as probsp,
            tc.tile_pool(name="ptp", bufs=3) as ptp,
            tc.tile_pool(name="ssp", bufs=2) as ssp,
            tc.tile_pool(name="op", bufs=3) as opool,
            tc.tile_pool(name="lnp", bufs=4) as lnp,
            tc.tile_pool(name="ps_a", bufs=3, space="PSUM") as ps_a,
            tc.tile_pool(name="ps_b", bufs=3, space="PSUM") as ps_b,
            tc.tile_pool(name="ps_tr", bufs=2, space="PSUM") as ps_tr,
        ):
            g.bpool, g.entp, g.entbp, g.enttp = bpool, entp, entbp, enttp
            g.ktp, g.kp, g.qinp, g.probsp, g.ptp = ktp, kp, qinp, probsp, ptp
            g.ssp, g.opool, g.lnp = ssp, opool, lnp
            g.ps_a, g.ps_b, g.ps_tr = ps_a, ps_b, ps_tr

            ident = wpool.tile([P, P], F32)
            make_identity(nc, ident)
            g.ident_b = wpool.tile([P, P], BF16)
            nc.vector.tensor_copy(g.ident_b, ident)
            g.ones_b = wpool.tile([1, P], BF16)
            nc.vector.memset(g.ones_b, 1.0)

            # batch-0 gathers go first on the SWDGE ring
            ent0, mbr0, g0_insts = _emit_gather(nc, g, 0)

            # weights: m1 first (K' needs it earliest), then m2, in column
            # halves so consumers can start after half arrives
            g.m1_sb = wpool.tile([P, DT, D], BF16)
            m1_r = m1[:, :].rearrange("(kt p) m -> p kt m", p=P)
            g.m2_sb = wpool.tile([P, DT, D], BF16)
            m2_r = m2[:, :].rearrange("(kt p) m -> p kt m", p=P)
            bulk = []
            for h in range(2):
                bulk.append(nc.scalar.dma_start(
                    g.m1_sb[:, :, h * LC:(h + 1) * LC],
                    m1_r[:, :, h * LC:(h + 1) * LC]))
            for h in range(2):
                bulk.append(nc.scalar.dma_start(
                    g.m2_sb[:, :, h * LC:(h + 1) * LC],
                    m2_r[:, :, h * LC:(h + 1) * LC]))
            for bi in bulk:
                for gi in g0_insts:
                    add_dep_helper(bi.ins, gi.ins,
                                   reason="bulk weight load after b0 gather")
            g.w_last = bulk[-1]

            g.m3_col = wpool.tile([P, DT], BF16)
            nc.scalar.dma_start(g.m3_col, m3[:].rearrange("(t p) -> p t", p=P))
            g.vb_bc = wpool.tile([P, D], F32)
            nc.scalar.dma_start(g.vb_bc, bcast_row(vb))
            if apply_affine:
                g.lng_bc = wpool.tile([P, D], F32)
                nc.scalar.dma_start(g.lng_bc, bcast_row(lng))
                g.lnb_bc = wpool.tile([P, D], F32)
                nc.scalar.dma_start(g.lnb_bc, bcast_row(lnb))

            state = {}

            def post_qin_hook(qin_i):
                # b1 gathers: after first query chunk on the DMA resource
                ent1, mbr1, g1_insts = _emit_gather(nc, g, 1)
                for gi in g1_insts:
                    add_dep_helper(gi.ins, qin_i.ins,
                                   reason="b1 gather after first query chunk")
                g.g1_insts = g1_insts
                state["b1"] = (ent1, mbr1)

            _emit_batch(nc, g, 0, ent0, mbr0, post_qin_hook=post_qin_hook)
            ent1, mbr1 = state["b1"]
            _emit_batch(nc, g, 1, ent1, mbr1)

    nc.compile()
    return nc


def _get_nc(apply_affine):
    key = bool(apply_affine)
    if key not in _CACHE:
        _CACHE[key] = build_nc(key)
    return _CACHE[key]


def kernel(query, ent_emb, ent_idx_in_batch, max_entity_number,
           WQ_w, WQ_b, WK_w, WK_b, WO_w, WO_b, ln_g, ln_b):
    query = np.asarray(query, np.float32)
    ent_emb = np.ascontiguousarray(np.asarray(ent_emb, np.float32))
    idx = np.asarray(ent_idx_in_batch)
    # additive pad-mask row: 0 for real entities, -1e5 for -1 padding
    mbias = ((idx != -1).astype(np.float32) - 1.0) * 1.0e5
    idx32 = np.where(idx < 0, 0, idx).astype(np.int32)
    wq = np.asarray(WQ_w, np.float32)
    wk = np.asarray(WK_w, np.float32)
    wo = np.asarray(WO_w, np.float32)
    bq = np.asarray(WQ_b, np.float32)
    bk = np.asarray(WK_b, np.float32)
    bo = np.asarray(WO_b, np.float32)
    lng = np.asarray(ln_g, np.float32)
    lnb = np.asarray(ln_b, np.float32)
    apply_affine = not (np.all(lng == 1.0) and np.all(lnb == 0.0))

    # host-folded fused weights (see module docstring)
    wkT = np.ascontiguousarray(wk.T)
    m1 = (wkT @ wq).astype(ml_dtypes.bfloat16)
    m2 = (wkT @ wo.T).astype(ml_dtypes.bfloat16)
    m3 = (wkT @ bq).astype(ml_dtypes.bfloat16)
    vb = (np.sqrt(float(D)) * bo + (bk @ wo.T)).astype(np.float32)

    qT = np.ascontiguousarray(
        query.transpose(0, 2, 1)).astype(ml_dtypes.bfloat16)  # (B, D, L)

    nc = _get_nc(apply_affine)
    in_maps = []
    for c in range(NCORES):
        s = slice(c * BL, (c + 1) * BL)
        m = dict(
            qT=np.ascontiguousarray(qT[s]),
            emb=ent_emb,
            idx=np.ascontiguousarray(idx32[s]),
            msk=np.ascontiguousarray(mbias[s]),
            m1=m1, m2=m2, m3=m3, vb=vb,
        )
        if apply_affine:
            m["lng"] = lng
            m["lnb"] = lnb
        in_maps.append(m)

    res = run_bass_kernel_spmd(nc, in_maps, core_ids=list(range(NCORES)))
    return np.concatenate([r["out"] for r in res.results], axis=0)
